# revision 1
# baseline (speedup 1.0000x reference)
"""Trainium2 Bass kernel for nn_CrossAttentionMasked.

Reference computation (B=4, N=4096, M=1024, QD=640, KD=VD=768, H=8, C=80):
    q = x @ Wq; k = key @ Wk; v = value @ Wv       (per-head C=80)
    S = q k^T / sqrt(C); qmask = box_mask.reshape(B,N) > 0.5
    S masked rows -> uniform softmax, but post-attention masked_fill zeroes
    those rows anyway, so masked rows' output is exactly `bout`.
    out = softmax(S) @ v  (rows zeroed where ~qmask); y = out @ Wout + bout

Sharding: 8 cores = 4 batches x 2 head-halves (4 heads per core).
Host compacts unmasked query rows (~50% of 4096) and transposes activations;
device computes projections, attention with S stored transposed ([m, n]
layout so no on-chip transposes are needed), softmax denominator via a
ones-column appended to V, and the output projection with bias folded in as
a ones-row of outT x a bias-row of Wout. Host sums the two head-half partial
outputs per batch and scatters into the full [B, N, QD] result.
"""

import os
from contextlib import ExitStack

import numpy as np

import concourse.bass as bass
import concourse.mybir as mybir
import concourse.tile as tile
from concourse import bacc
from concourse.bass_utils import run_bass_kernel_spmd

B, N, M = 4, 4096, 1024
QD, KD, VD = 640, 768, 768
H, C = 8, 80
SIZE = 64
HPC = 4            # heads per core
CP = 128           # per-head channel dim padded 80 -> 128
VAUG = 97          # v chans + 16 zero pad + ones col at 96 (32-aligned)
SCALE = C ** -0.5
F32 = mybir.dt.float32
F32R = mybir.dt.float32r
EXP = mybir.ActivationFunctionType.Exp
COPY = mybir.ActivationFunctionType.Copy
MUL = mybir.AluOpType.mult

DQ_CH = QD // 128  # 5
DK_CH = KD // 128  # 6
M_CH = M // 128    # 8
WOUT_ROWS = 321    # 4 heads x 80 rows + 1 bias row


def _head_segments(h):
    """outT row segments for head h (rows 80h..80h+80 split at 128-chunk
    boundaries) as (chunk, chunk_row, c0, c1)."""
    segs, c = [], 0
    while c < 80:
        r = 80 * h + c
        j, p = r // 128, r % 128
        span = min(80 - c, 128 - p)
        segs.append((j, p, c, c + span))
        c += span
    return segs


def _pbcast(row_ap, nparts):
    """Partition-broadcast AP: replicate a [1, F] DRAM row across nparts."""
    return bass.AP(tensor=row_ap.tensor, offset=row_ap.offset,
                   ap=[[0, nparts], list(row_ap.ap[-1])])


def build(npad):
    """Build the per-core Bass program for NPAD compacted+padded queries."""
    nc = bacc.Bacc("TRN2", target_bir_lowering=False)

    xt = nc.dram_tensor("xt", [QD, npad], F32R, kind="ExternalInput")
    kt = nc.dram_tensor("kt", [KD, M], F32R, kind="ExternalInput")
    vt = nc.dram_tensor("vt", [KD, M], F32R, kind="ExternalInput")
    wq = nc.dram_tensor("wq", [QD, HPC * CP], F32R, kind="ExternalInput")
    wk = nc.dram_tensor("wk", [KD, HPC * CP], F32R, kind="ExternalInput")
    wv = nc.dram_tensor("wv", [KD, HPC * VAUG], F32R, kind="ExternalInput")
    wout = nc.dram_tensor("wout", [WOUT_ROWS, QD], F32R, kind="ExternalInput")
    y = nc.dram_tensor("y", [npad, QD], F32, kind="ExternalOutput")

    xt_r = xt.rearrange("(dc p) n -> p dc n", p=128)
    kt_r = kt.rearrange("(dc p) m -> p dc m", p=128)
    vt_r = vt.rearrange("(dc p) m -> p dc m", p=128)
    wq_r = wq.rearrange("(dc p) e -> p dc e", p=128)
    wk_r = wk.rearrange("(dc p) e -> p dc e", p=128)
    wv_r = wv.rearrange("(dc p) e -> p dc e", p=128)
    y_r = y.rearrange("(nt p) d -> p nt d", p=128)

    n_tiles = npad // 128
    # n groups of up to 512; avoid a trailing group < 256 (fp32r matmuls
    # with free dim < 256 drop to 1/4 rate)
    groups = []
    off = 0
    while off < npad:
        rem = npad - off
        if rem > 640 or rem <= 512:
            g = min(512, rem)
        else:
            g = 384  # leaves rem-384 in [128+128, 256]: 256 next
        groups.append((off, g))
        off += g

    with TileKernel(nc) as emit:
        emit(xt_r, kt_r, vt_r, wq_r, wk_r, wv_r, wout, y_r,
             groups, n_tiles, npad)
    nc.compile()
    return nc


class TileKernel:
    def __init__(self, nc):
        self.nc = nc
        self.ctx = ExitStack()

    def __enter__(self):
        self.tc = self.ctx.enter_context(tile.TileContext(self.nc))
        return self.emit

    def __exit__(self, *exc):
        return self.ctx.__exit__(*exc)

    def emit(self, xt_r, kt_r, vt_r, wq_r, wk_r, wv_r, wout, y_r,
             groups, n_tiles, npad):
        nc, tc, ctx = self.nc, self.tc, self.ctx

        res = ctx.enter_context(tc.tile_pool(name="resident", bufs=1))
        # persistent tensors
        q_heads = [res.tile([128, npad], F32R, tag=f"qT{h}", name=f"qT{h}") for h in range(HPC)]
        k_heads = [res.tile([128, M], F32R, tag=f"kT{h}", name=f"kT{h}") for h in range(HPC)]
        v_sb = res.tile([128, M_CH, HPC * VAUG], F32R, tag="v_sb", name="v_sb")
        wout_hs = []
        for h in range(HPC):
            w = res.tile([80, QD], F32R, tag=f"woutH{h}", name=f"woutH{h}")
            nc.sync.dma_start(out=w[:], in_=wout[h * 80:(h + 1) * 80, :])
            wout_hs.append(w)
        bias_bc = res.tile([128, QD], F32R, tag="bias_bc", name="bias_bc")
        nc.gpsimd.dma_start(
            out=bias_bc[:],
            in_=_pbcast(wout[WOUT_ROWS - 1:WOUT_ROWS, :], 128))

        # ---- phase P: projections ----
        with (
            tc.tile_pool(name="wpool", bufs=1) as wp,
            tc.tile_pool(name="pin", bufs=3) as pin,
            tc.tile_pool(name="ppsum", bufs=4, space="PSUM") as pps,
        ):
            wq_sb = wp.tile([128, DQ_CH, HPC * CP], F32R, tag="wq_sb", name="wq_sb")
            wk_sb = wp.tile([128, DK_CH, HPC * CP], F32R, tag="wk_sb", name="wk_sb")
            wv_sb = wp.tile([128, DK_CH, HPC * VAUG], F32R, tag="wv_sb", name="wv_sb")
            nc.sync.dma_start(out=wq_sb[:], in_=wq_r[:])
            nc.sync.dma_start(out=wk_sb[:], in_=wk_r[:])
            nc.sync.dma_start(out=wv_sb[:], in_=wv_r[:])

            # qT[h] = (x @ Wq_h)^T computed as Wq_h^T-free: lhsT=wq chunk
            for g0, gsz in groups:
                xq = pin.tile([128, DQ_CH, 512], F32R, tag="xq", name="xq")
                nc.sync.dma_start(out=xq[:, :, :gsz],
                                  in_=xt_r[:, :, g0:g0 + gsz])
                for h in range(HPC):
                    ps = pps.tile([128, 512], F32, tag="pp", name="pp")
                    for dc in range(DQ_CH):
                        nc.tensor.matmul(
                            ps[:, :gsz],
                            wq_sb[:, dc, h * CP:(h + 1) * CP],
                            xq[:, dc, :gsz],
                            start=(dc == 0), stop=(dc == DQ_CH - 1))
                    nc.scalar.activation(q_heads[h][:, g0:g0 + gsz],
                                         ps[:, :gsz], COPY)

            # kT[h] = (key @ Wk_h)^T
            for mg in range(2):
                ksl = pin.tile([128, DK_CH, 512], F32R, tag="ksl", name="ksl")
                nc.sync.dma_start(out=ksl[:],
                                  in_=kt_r[:, :, mg * 512:(mg + 1) * 512])
                for h in range(HPC):
                    ps = pps.tile([128, 512], F32, tag="pp", name="pp")
                    for dc in range(DK_CH):
                        nc.tensor.matmul(
                            ps[:],
                            wk_sb[:, dc, h * CP:(h + 1) * CP],
                            ksl[:, dc, :],
                            start=(dc == 0), stop=(dc == DK_CH - 1))
                    nc.scalar.activation(
                        k_heads[h][:, mg * 512:(mg + 1) * 512], ps[:], COPY)

            # v natural layout [m, head-aug channels]; ones col per head
            for mc in range(M_CH):
                vsl = pin.tile([128, DK_CH, 128], F32R, tag="vsl", name="vsl")
                nc.sync.dma_start(out=vsl[:],
                                  in_=vt_r[:, :, mc * 128:(mc + 1) * 128])
                ps = pps.tile([128, HPC * VAUG], F32, tag="pp", name="pp")
                for dc in range(DK_CH):
                    nc.tensor.matmul(
                        ps[:], vsl[:, dc, :], wv_sb[:, dc, :],
                        start=(dc == 0), stop=(dc == DK_CH - 1))
                nc.scalar.activation(v_sb[:, mc, :], ps[:], COPY)
                for h in range(HPC):
                    nc.vector.tensor_copy(
                        v_sb[:, mc, h * VAUG + 96:h * VAUG + 97],
                        nc.const_aps.tensor(1.0, (128, 1), F32))

        # ---- phase A: attention + output projection ----
        if os.environ.get("KERNEL_SKIP_ATTN"):
            return
        with (
            tc.tile_pool(name="apool", bufs=2) as ap,
            tc.tile_pool(name="stp", bufs=4, space="PSUM") as stp,
            tc.tile_pool(name="ovp", bufs=2, space="PSUM") as ovp,
            tc.tile_pool(name="yp", bufs=1, space="PSUM") as yp,
            tc.tile_pool(name="dsc", bufs=3, space="DRAM") as dsc,
        ):
            for g0, gsz in groups:
                out_hs = []
                for h in range(HPC):
                    expst = ap.tile([128, M_CH, 512], F32R, tag="expst",
                                    name="expst")
                    for mc in range(M_CH):
                        st = stp.tile([128, 512], F32, tag="st", name="st")
                        nc.tensor.matmul(
                            st[:, :gsz],
                            k_heads[h][:, mc * 128:(mc + 1) * 128],
                            q_heads[h][:, g0:g0 + gsz],
                            start=True, stop=True)
                        nc.scalar.activation(expst[:, mc, :gsz], st[:, :gsz],
                                             EXP, scale=SCALE)
                    oaug = ovp.tile([VAUG, 512], F32, tag="oaug", name="oaug")
                    for mc in range(M_CH):
                        nc.tensor.matmul(
                            oaug[:, :gsz],
                            v_sb[:, mc, h * VAUG:(h + 1) * VAUG],
                            expst[:, mc, :gsz],
                            start=(mc == 0), stop=(mc == M_CH - 1))
                    recip = ap.tile([1, 512], F32, tag="recip", name="recip")
                    nc.vector.reciprocal(recip[:, :gsz], oaug[96:97, :gsz])
                    rdr = dsc.tile([1, 512], F32, tag="rdr", name="rdr")
                    nc.sync.dma_start(out=rdr[:, :gsz], in_=recip[:, :gsz])
                    bcast = ap.tile([80, 512], F32, tag="bcast", name="bcast")
                    nc.gpsimd.dma_start(out=bcast[:, :gsz],
                                        in_=_pbcast(rdr[:1, :gsz], 80))
                    out_h = ap.tile([80, 512], F32R, tag=f"outH{h}",
                                    name=f"outH{h}")
                    nc.vector.tensor_tensor(
                        out_h[:, :gsz], oaug[:80, :gsz], bcast[:, :gsz], MUL)
                    out_hs.append(out_h)

                for nt0 in range(gsz // 128):
                    nt = g0 // 128 + nt0
                    yps = yp.tile([128, 2, 512], F32, tag="y", name="y")
                    for di in range(2):
                        for h in range(HPC):
                            nc.tensor.matmul(
                                yps[:, di, :320],
                                out_hs[h][:, nt0 * 128:(nt0 + 1) * 128],
                                wout_hs[h][:, di * 320:(di + 1) * 320],
                                start=(h == 0), stop=(h == HPC - 1))
                    ysb = ap.tile([128, QD], F32, tag="ysb", name="ysb")
                    for di in range(2):
                        nc.vector.tensor_tensor(
                            ysb[:, di * 320:(di + 1) * 320],
                            yps[:, di, :320],
                            bias_bc[:, di * 320:(di + 1) * 320],
                            mybir.AluOpType.add)
                    nc.sync.dma_start(out=y_r[:, nt, :], in_=ysb[:])


def _prep_core_inputs(x, key, value, wq, wk, wv, wout, bout,
                      qmask_idx, npad):
    """Host-side shard prep: returns list of 8 in_maps."""
    f32 = np.float32
    in_maps = []
    xt_b, kt_b, vt_b = {}, {}, {}
    for b in range(B):
        idx = qmask_idx[b]
        xs = np.zeros((QD, npad), dtype=f32)
        xs[:, :len(idx)] = np.ascontiguousarray(x[b][idx].T)
        xt_b[b] = xs
        kt_b[b] = np.ascontiguousarray(key[b].T).astype(f32)
        vt_b[b] = np.ascontiguousarray(value[b].T).astype(f32)

    w_half = {}
    for hh in range(2):
        wq_a = np.zeros((QD, HPC * CP), dtype=f32)
        wk_a = np.zeros((KD, HPC * CP), dtype=f32)
        wv_a = np.zeros((KD, HPC * VAUG), dtype=f32)
        for hp in range(HPC):
            hg = hh * HPC + hp
            wq_a[:, hp * CP:hp * CP + 80] = wq[:, hg * 80:(hg + 1) * 80]
            wk_a[:, hp * CP:hp * CP + 80] = wk[:, hg * 80:(hg + 1) * 80]
            wv_a[:, hp * VAUG:hp * VAUG + 80] = wv[:, hg * 80:(hg + 1) * 80]
        w_half[hh] = (wq_a, wk_a, wv_a)

    for core in range(8):
        b, hh = core // 2, core % 2
        wq_a, wk_a, wv_a = w_half[hh]
        wout_a = np.zeros((WOUT_ROWS, QD), dtype=f32)
        wout_a[:HPC * 80] = wout[hh * HPC * 80:(hh + 1) * HPC * 80]
        if hh == 0:
            wout_a[HPC * 80] = bout  # bias row, broadcast-added on device
        in_maps.append({
            "xt": xt_b[b], "kt": kt_b[b], "vt": vt_b[b],
            "wq": wq_a, "wk": wk_a, "wv": wv_a, "wout": wout_a,
        })
    return in_maps


def kernel(x, key, value, box_mask, Wq, Wk, Wv, Wout, bout, _trace=False):
    x = np.asarray(x, dtype=np.float32)
    key = np.asarray(key, dtype=np.float32)
    value = np.asarray(value, dtype=np.float32)
    box_mask = np.asarray(box_mask, dtype=np.float32)
    Wq, Wk, Wv = (np.asarray(a, dtype=np.float32) for a in (Wq, Wk, Wv))
    Wout = np.asarray(Wout, dtype=np.float32)
    bout = np.asarray(bout, dtype=np.float32)

    qmask = box_mask[:, 0].reshape(B, N) > 0.5
    qmask_idx = [np.nonzero(qmask[b])[0] for b in range(B)]
    cnt_max = max(1, max(len(i) for i in qmask_idx))
    npad = -(-cnt_max // 128) * 128

    nc = build(npad)
    in_maps = _prep_core_inputs(x, key, value, Wq, Wk, Wv, Wout, bout,
                                qmask_idx, npad)
    kr = run_bass_kernel_spmd(nc, in_maps, core_ids=list(range(8)),
                              trace=_trace)
    results = kr.results

    out = np.broadcast_to(bout, (B, N, QD)).copy().astype(np.float32)
    for b in range(B):
        idx = qmask_idx[b]
        yb = results[2 * b]["y"][:len(idx)] + results[2 * b + 1]["y"][:len(idx)]
        out[b][idx] = yb
    if _trace:
        return out, kr
    return out



# revision 5
# speedup vs baseline: 557.9568x; 557.9568x over previous
"""Trainium2 Bass kernel for nn_CrossAttentionMasked.

Reference computation (B=4, N=4096, M=1024, QD=640, KD=VD=768, H=8, C=80):
    q = x @ Wq; k = key @ Wk; v = value @ Wv       (per-head C=80)
    S = q k^T / sqrt(C); qmask = box_mask.reshape(B,N) > 0.5
    S masked rows -> uniform softmax, but post-attention masked_fill zeroes
    those rows anyway, so masked rows' output is exactly `bout`.
    out = softmax(S) @ v  (rows zeroed where ~qmask); y = out @ Wout + bout

Sharding: 8 cores = 4 batches x 2 head-halves (4 heads per core).
Host compacts unmasked query rows (~50% of 4096) and transposes activations;
device computes projections, attention with S stored transposed ([m, n]
layout so no on-chip transposes are needed), softmax denominator via a
ones-column appended to V, and the output projection with bias folded in as
a ones-row of outT x a bias-row of Wout. Host sums the two head-half partial
outputs per batch and scatters into the full [B, N, QD] result.
"""

import os
from contextlib import ExitStack

import numpy as np

import concourse.bass as bass
import concourse.mybir as mybir
import concourse.tile as tile
from concourse import bacc
from concourse.bass_utils import run_bass_kernel_spmd

B, N, M = 4, 4096, 1024
QD, KD, VD = 640, 768, 768
H, C = 8, 80
SIZE = 64
HPC = 4            # heads per core
CP = 128           # per-head channel dim padded 80 -> 128
VAUG = 97          # v chans + 16 zero pad + ones col at 96 (32-aligned)
SCALE = C ** -0.5
F32 = mybir.dt.float32
F32R = mybir.dt.float32r
EXP = mybir.ActivationFunctionType.Exp
COPY = mybir.ActivationFunctionType.Copy
MUL = mybir.AluOpType.mult

DQ_CH = QD // 128  # 5
DK_CH = KD // 128  # 6
M_CH = M // 128    # 8
WOUT_ROWS = 321    # 4 heads x 80 rows + 1 bias row


def _head_segments(h):
    """outT row segments for head h (rows 80h..80h+80 split at 128-chunk
    boundaries) as (chunk, chunk_row, c0, c1)."""
    segs, c = [], 0
    while c < 80:
        r = 80 * h + c
        j, p = r // 128, r % 128
        span = min(80 - c, 128 - p)
        segs.append((j, p, c, c + span))
        c += span
    return segs


def _pbcast(row_ap, nparts):
    """Partition-broadcast AP: replicate a [1, F] DRAM row across nparts."""
    return bass.AP(tensor=row_ap.tensor, offset=row_ap.offset,
                   ap=[[0, nparts], list(row_ap.ap[-1])])


def build(npad, reps=1):
    """Build the per-core Bass program for NPAD compacted+padded queries.

    reps > 1 wraps the whole body in a hardware loop that re-runs the full
    computation (idempotent: same DRAM in/out each iteration) — used by the
    timing harness to amortize the fixed per-dispatch RPC overhead out of
    the hardware-time measurement.
    """
    nc = bacc.Bacc("TRN2", target_bir_lowering=False)

    xt = nc.dram_tensor("xt", [QD, npad], F32R, kind="ExternalInput")
    kt = nc.dram_tensor("kt", [KD, M], F32R, kind="ExternalInput")
    vt = nc.dram_tensor("vt", [KD, M], F32R, kind="ExternalInput")
    wq = nc.dram_tensor("wq", [QD, HPC * CP], F32R, kind="ExternalInput")
    wk = nc.dram_tensor("wk", [KD, HPC * CP], F32R, kind="ExternalInput")
    wv = nc.dram_tensor("wv", [KD, HPC * VAUG], F32R, kind="ExternalInput")
    wout = nc.dram_tensor("wout", [WOUT_ROWS, QD], F32R, kind="ExternalInput")
    y = nc.dram_tensor("y", [npad, QD], F32, kind="ExternalOutput")

    xt_r = xt.rearrange("(dc p) n -> p dc n", p=128)
    kt_r = kt.rearrange("(dc p) m -> p dc m", p=128)
    vt_r = vt.rearrange("(dc p) m -> p dc m", p=128)
    wq_r = wq.rearrange("(dc p) e -> p dc e", p=128)
    wk_r = wk.rearrange("(dc p) e -> p dc e", p=128)
    wv_r = wv.rearrange("(dc p) e -> p dc e", p=128)
    y_r = y.rearrange("(nt p) d -> p nt d", p=128)

    n_tiles = npad // 128
    # n groups of up to 512; avoid a trailing group < 256 (fp32r matmuls
    # with free dim < 256 drop to 1/4 rate)
    groups = []
    off = 0
    while off < npad:
        rem = npad - off
        if rem > 640 or rem <= 512:
            g = min(512, rem)
        else:
            g = 384  # leaves rem-384 in [128+128, 256]: 256 next
        groups.append((off, g))
        off += g

    with TileKernel(nc) as tk:
        if reps > 1:
            with tk.tc.For_i(0, reps, 1,
                             hint_engines=(mybir.EngineType.PE,)):
                tk.emit(xt_r, kt_r, vt_r, wq_r, wk_r, wv_r, wout, y_r,
                        groups, n_tiles, npad)
        else:
            tk.emit(xt_r, kt_r, vt_r, wq_r, wk_r, wv_r, wout, y_r,
                    groups, n_tiles, npad)
    nc.compile()
    return nc


class TileKernel:
    def __init__(self, nc):
        self.nc = nc
        self.ctx = ExitStack()

    def __enter__(self):
        self.tc = self.ctx.enter_context(tile.TileContext(self.nc))
        return self

    def __exit__(self, *exc):
        return self.ctx.__exit__(*exc)

    def emit(self, xt_r, kt_r, vt_r, wq_r, wk_r, wv_r, wout, y_r,
             groups, n_tiles, npad):
        nc, tc, ctx = self.nc, self.tc, self.ctx

        res = ctx.enter_context(tc.tile_pool(name="resident", bufs=1))
        # persistent tensors
        q_heads = [res.tile([128, npad], F32R, tag=f"qT{h}", name=f"qT{h}") for h in range(HPC)]
        k_heads = [res.tile([128, M], F32R, tag=f"kT{h}", name=f"kT{h}") for h in range(HPC)]
        v_sb = res.tile([128, M_CH, HPC * VAUG], F32R, tag="v_sb", name="v_sb")
        wout_hs = []
        for h in range(HPC):
            w = res.tile([80, QD], F32R, tag=f"woutH{h}", name=f"woutH{h}")
            nc.sync.dma_start(out=w[:], in_=wout[h * 80:(h + 1) * 80, :])
            wout_hs.append(w)
        bias_bc = res.tile([128, QD], F32R, tag="bias_bc", name="bias_bc")
        nc.gpsimd.dma_start(
            out=bias_bc[:],
            in_=_pbcast(wout[WOUT_ROWS - 1:WOUT_ROWS, :], 128))

        # ---- phase P: projections ----
        with (
            tc.tile_pool(name="wpool", bufs=1) as wp,
            tc.tile_pool(name="pin", bufs=3) as pin,
            tc.tile_pool(name="ppsum", bufs=4, space="PSUM") as pps,
        ):
            wq_sb = wp.tile([128, DQ_CH, HPC * CP], F32R, tag="wq_sb", name="wq_sb")
            wk_sb = wp.tile([128, DK_CH, HPC * CP], F32R, tag="wk_sb", name="wk_sb")
            wv_sb = wp.tile([128, DK_CH, HPC * VAUG], F32R, tag="wv_sb", name="wv_sb")
            nc.sync.dma_start(out=wq_sb[:], in_=wq_r[:])
            nc.sync.dma_start(out=wk_sb[:], in_=wk_r[:])
            nc.sync.dma_start(out=wv_sb[:], in_=wv_r[:])

            # qT[h] = (x @ Wq_h)^T computed as Wq_h^T-free: lhsT=wq chunk
            for g0, gsz in groups:
                xq = pin.tile([128, DQ_CH, 512], F32R, tag="xq", name="xq")
                nc.sync.dma_start(out=xq[:, :, :gsz],
                                  in_=xt_r[:, :, g0:g0 + gsz])
                for h in range(HPC):
                    ps = pps.tile([128, 512], F32, tag="pp", name="pp")
                    for dc in range(DQ_CH):
                        nc.tensor.matmul(
                            ps[:, :gsz],
                            wq_sb[:, dc, h * CP:(h + 1) * CP],
                            xq[:, dc, :gsz],
                            start=(dc == 0), stop=(dc == DQ_CH - 1))
                    nc.scalar.activation(q_heads[h][:, g0:g0 + gsz],
                                         ps[:, :gsz], COPY)

            # kT[h] = (key @ Wk_h)^T
            for mg in range(2):
                ksl = pin.tile([128, DK_CH, 512], F32R, tag="ksl", name="ksl")
                nc.sync.dma_start(out=ksl[:],
                                  in_=kt_r[:, :, mg * 512:(mg + 1) * 512])
                for h in range(HPC):
                    ps = pps.tile([128, 512], F32, tag="pp", name="pp")
                    for dc in range(DK_CH):
                        nc.tensor.matmul(
                            ps[:],
                            wk_sb[:, dc, h * CP:(h + 1) * CP],
                            ksl[:, dc, :],
                            start=(dc == 0), stop=(dc == DK_CH - 1))
                    nc.scalar.activation(
                        k_heads[h][:, mg * 512:(mg + 1) * 512], ps[:], COPY)

            # v natural layout [m, head-aug channels]; ones col per head
            for mc in range(M_CH):
                vsl = pin.tile([128, DK_CH, 128], F32R, tag="vsl", name="vsl")
                nc.sync.dma_start(out=vsl[:],
                                  in_=vt_r[:, :, mc * 128:(mc + 1) * 128])
                ps = pps.tile([128, HPC * VAUG], F32, tag="pp", name="pp")
                for dc in range(DK_CH):
                    nc.tensor.matmul(
                        ps[:], vsl[:, dc, :], wv_sb[:, dc, :],
                        start=(dc == 0), stop=(dc == DK_CH - 1))
                nc.scalar.activation(v_sb[:, mc, :], ps[:], COPY)
                for h in range(HPC):
                    nc.vector.tensor_copy(
                        v_sb[:, mc, h * VAUG + 96:h * VAUG + 97],
                        nc.const_aps.tensor(1.0, (128, 1), F32))

        # ---- phase A: attention + output projection ----
        if os.environ.get("KERNEL_SKIP_ATTN"):
            return
        with (
            tc.tile_pool(name="apool", bufs=2) as ap,
            tc.tile_pool(name="stp", bufs=4, space="PSUM") as stp,
            tc.tile_pool(name="ovp", bufs=2, space="PSUM") as ovp,
            tc.tile_pool(name="yp", bufs=1, space="PSUM") as yp,
            tc.tile_pool(name="dsc", bufs=3, space="DRAM") as dsc,
        ):
            for g0, gsz in groups:
                out_hs = []
                for h in range(HPC):
                    expst = ap.tile([128, M_CH, 512], F32R, tag="expst",
                                    name="expst")
                    for mc in range(M_CH):
                        st = stp.tile([128, 512], F32, tag="st", name="st")
                        nc.tensor.matmul(
                            st[:, :gsz],
                            k_heads[h][:, mc * 128:(mc + 1) * 128],
                            q_heads[h][:, g0:g0 + gsz],
                            start=True, stop=True)
                        nc.scalar.activation(expst[:, mc, :gsz], st[:, :gsz],
                                             EXP, scale=SCALE)
                    oaug = ovp.tile([VAUG, 512], F32, tag="oaug", name="oaug")
                    for mc in range(M_CH):
                        nc.tensor.matmul(
                            oaug[:, :gsz],
                            v_sb[:, mc, h * VAUG:(h + 1) * VAUG],
                            expst[:, mc, :gsz],
                            start=(mc == 0), stop=(mc == M_CH - 1))
                    recip = ap.tile([1, 512], F32, tag="recip", name="recip")
                    nc.vector.reciprocal(recip[:, :gsz], oaug[96:97, :gsz])
                    rdr = dsc.tile([1, 512], F32, tag="rdr", name="rdr")
                    nc.sync.dma_start(out=rdr[:, :gsz], in_=recip[:, :gsz])
                    bcast = ap.tile([80, 512], F32, tag="bcast", name="bcast")
                    nc.gpsimd.dma_start(out=bcast[:, :gsz],
                                        in_=_pbcast(rdr[:1, :gsz], 80))
                    out_h = ap.tile([80, 512], F32R, tag=f"outH{h}",
                                    name=f"outH{h}")
                    nc.vector.tensor_tensor(
                        out_h[:, :gsz], oaug[:80, :gsz], bcast[:, :gsz], MUL)
                    out_hs.append(out_h)

                for nt0 in range(gsz // 128):
                    nt = g0 // 128 + nt0
                    yps = yp.tile([128, 2, 512], F32, tag="y", name="y")
                    for di in range(2):
                        for h in range(HPC):
                            nc.tensor.matmul(
                                yps[:, di, :320],
                                out_hs[h][:, nt0 * 128:(nt0 + 1) * 128],
                                wout_hs[h][:, di * 320:(di + 1) * 320],
                                start=(h == 0), stop=(h == HPC - 1))
                    ysb = ap.tile([128, QD], F32, tag="ysb", name="ysb")
                    for di in range(2):
                        nc.vector.tensor_tensor(
                            ysb[:, di * 320:(di + 1) * 320],
                            yps[:, di, :320],
                            bias_bc[:, di * 320:(di + 1) * 320],
                            mybir.AluOpType.add)
                    nc.sync.dma_start(out=y_r[:, nt, :], in_=ysb[:])


def _prep_core_inputs(x, key, value, wq, wk, wv, wout, bout,
                      qmask_idx, npad):
    """Host-side shard prep: returns list of 8 in_maps."""
    f32 = np.float32
    in_maps = []
    xt_b, kt_b, vt_b = {}, {}, {}
    for b in range(B):
        idx = qmask_idx[b]
        xs = np.zeros((QD, npad), dtype=f32)
        xs[:, :len(idx)] = np.ascontiguousarray(x[b][idx].T)
        xt_b[b] = xs
        kt_b[b] = np.ascontiguousarray(key[b].T).astype(f32)
        vt_b[b] = np.ascontiguousarray(value[b].T).astype(f32)

    w_half = {}
    for hh in range(2):
        wq_a = np.zeros((QD, HPC * CP), dtype=f32)
        wk_a = np.zeros((KD, HPC * CP), dtype=f32)
        wv_a = np.zeros((KD, HPC * VAUG), dtype=f32)
        for hp in range(HPC):
            hg = hh * HPC + hp
            wq_a[:, hp * CP:hp * CP + 80] = wq[:, hg * 80:(hg + 1) * 80]
            wk_a[:, hp * CP:hp * CP + 80] = wk[:, hg * 80:(hg + 1) * 80]
            wv_a[:, hp * VAUG:hp * VAUG + 80] = wv[:, hg * 80:(hg + 1) * 80]
        w_half[hh] = (wq_a, wk_a, wv_a)

    for core in range(8):
        b, hh = core // 2, core % 2
        wq_a, wk_a, wv_a = w_half[hh]
        wout_a = np.zeros((WOUT_ROWS, QD), dtype=f32)
        wout_a[:HPC * 80] = wout[hh * HPC * 80:(hh + 1) * HPC * 80]
        if hh == 0:
            wout_a[HPC * 80] = bout  # bias row, broadcast-added on device
        in_maps.append({
            "xt": xt_b[b], "kt": kt_b[b], "vt": vt_b[b],
            "wq": wq_a, "wk": wk_a, "wv": wv_a, "wout": wout_a,
        })
    return in_maps


def kernel(x, key, value, box_mask, Wq, Wk, Wv, Wout, bout, _trace=False):
    x = np.asarray(x, dtype=np.float32)
    key = np.asarray(key, dtype=np.float32)
    value = np.asarray(value, dtype=np.float32)
    box_mask = np.asarray(box_mask, dtype=np.float32)
    Wq, Wk, Wv = (np.asarray(a, dtype=np.float32) for a in (Wq, Wk, Wv))
    Wout = np.asarray(Wout, dtype=np.float32)
    bout = np.asarray(bout, dtype=np.float32)

    qmask = box_mask[:, 0].reshape(B, N) > 0.5
    qmask_idx = [np.nonzero(qmask[b])[0] for b in range(B)]
    cnt_max = max(1, max(len(i) for i in qmask_idx))
    npad = -(-cnt_max // 128) * 128

    nc = build(npad)
    in_maps = _prep_core_inputs(x, key, value, Wq, Wk, Wv, Wout, bout,
                                qmask_idx, npad)
    kr = run_bass_kernel_spmd(nc, in_maps, core_ids=list(range(8)),
                              trace=_trace)
    results = kr.results

    out = np.broadcast_to(bout, (B, N, QD)).copy().astype(np.float32)
    for b in range(B):
        idx = qmask_idx[b]
        yb = results[2 * b]["y"][:len(idx)] + results[2 * b + 1]["y"][:len(idx)]
        out[b][idx] = yb
    if _trace:
        return out, kr
    return out



# revision 24
# speedup vs baseline: 905.3670x; 1.6226x over previous
"""Trainium2 Bass kernel for nn_CrossAttentionMasked.

Reference computation (B=4, N=4096, M=1024, QD=640, KD=VD=768, H=8, C=80):
    q = x @ Wq; k = key @ Wk; v = value @ Wv       (per-head C=80)
    S = q k^T / sqrt(C); qmask = box_mask.reshape(B,N) > 0.5
    S masked rows -> uniform softmax, but post-attention masked_fill zeroes
    those rows anyway, so masked rows' output is exactly `bout`.
    out = softmax(S) @ v  (rows zeroed where ~qmask); y = out @ Wout + bout

Sharding: 8 cores = 4 batches x 2 head-halves (4 heads per core).
Host compacts unmasked query rows (~50% of 4096) and transposes activations;
device computes projections, attention with S stored transposed ([m, n]
layout so no on-chip transposes are needed), softmax denominator via a
ones-column appended to V, and the output projection. Host sums the two
head-half partial outputs per batch and scatters into the full result.

All matmul operands are bf16 (fp32 PSUM accumulation), which halves HBM
traffic and SBUF footprint.  Every DRAM tensor is pre-laid-out partition-
major on the host so each DMA is one contiguous run per partition (128
descriptors instead of 640+ — descriptor generation was a bottleneck).
The kernel is one merged pipeline (k-proj, v-proj, then per query-group:
q-proj -> attention -> out-proj); k/v PSUM->SBUF copies run on the
otherwise-idle ACT engine during the projection prologue, q copies on DVE.
"""

from contextlib import ExitStack

import numpy as np
from ml_dtypes import bfloat16

import concourse.bass as bass
import concourse.mybir as mybir
import concourse.tile as tile
from concourse import bacc
from concourse.bass_utils import run_bass_kernel_spmd

B, N, M = 4, 4096, 1024
QD, KD, VD = 640, 768, 768
H, C = 8, 80
SIZE = 64
HPC = 4            # heads per core
CP = 128           # per-head channel dim padded 80 -> 128
VAUG = 97          # v chans (80) + zero pad + ones col at 96 (32-aligned
                   # partition base: engine APs must start at 0 mod 32)
SCALE = C ** -0.5
F32 = mybir.dt.float32
BF16 = mybir.dt.bfloat16
EXP = mybir.ActivationFunctionType.Exp
COPY = mybir.ActivationFunctionType.Copy
MUL = mybir.AluOpType.mult
ADD = mybir.AluOpType.add

DQ_CH = QD // 128  # 5
DK_CH = KD // 128  # 6
M_CH = M // 128    # 8


def _pbcast(row_ap, nparts):
    """Partition-broadcast AP: replicate a DRAM row-block across nparts."""
    return bass.AP(tensor=row_ap.tensor, offset=row_ap.offset,
                   ap=[[0, nparts]] + [list(d) for d in row_ap.ap])


def build(npad, reps=1):
    """Build the per-core Bass program for NPAD compacted+padded queries.

    reps > 1 wraps the whole body in a hardware loop that re-runs the full
    computation (idempotent: same DRAM in/out each iteration) — used by the
    timing harness to amortize the fixed per-dispatch RPC overhead out of
    the hardware-time measurement.
    """
    nc = bacc.Bacc("TRN2", target_bir_lowering=False)
    ngr = -(-npad // 512)  # query groups of <=512

    xt = nc.dram_tensor("xt", [128, ngr, DQ_CH, 512], BF16,
                        kind="ExternalInput")
    kt = nc.dram_tensor("kt", [128, 2, DK_CH, 512], BF16,
                        kind="ExternalInput")
    vt = nc.dram_tensor("vt", [128, M_CH, DK_CH, 128], BF16,
                        kind="ExternalInput")
    wq = nc.dram_tensor("wq", [128, DQ_CH, HPC * CP], BF16,
                        kind="ExternalInput")
    wk = nc.dram_tensor("wk", [128, DK_CH, HPC * CP], BF16,
                        kind="ExternalInput")
    wv = nc.dram_tensor("wv", [128, DK_CH, HPC * VAUG], BF16,
                        kind="ExternalInput")
    wout = nc.dram_tensor("wout", [80, HPC, QD], BF16, kind="ExternalInput")
    brow = nc.dram_tensor("brow", [1, QD], F32, kind="ExternalInput")
    y = nc.dram_tensor("y", [128, ngr, 4, QD], BF16, kind="ExternalOutput")

    groups = [(off, min(512, npad - off)) for off in range(0, npad, 512)]

    with TileKernel(nc) as tk:
        if reps > 1:
            with tk.tc.For_i(0, reps, 1,
                             hint_engines=(mybir.EngineType.PE,)):
                tk.emit(xt, kt, vt, wq, wk, wv, wout, brow, y, groups, npad)
        else:
            tk.emit(xt, kt, vt, wq, wk, wv, wout, brow, y, groups, npad)
    nc.compile()
    return nc


class TileKernel:
    def __init__(self, nc):
        self.nc = nc
        self.ctx = ExitStack()

    def __enter__(self):
        self.tc = self.ctx.enter_context(tile.TileContext(self.nc))
        return self

    def __exit__(self, *exc):
        return self.ctx.__exit__(*exc)

    def emit(self, xt, kt, vt, wq, wk, wv, wout, brow, y, groups, npad):
        nc, tc, ctx = self.nc, self.tc, self.ctx

        res = ctx.enter_context(tc.tile_pool(name="resident", bufs=1))
        pin = ctx.enter_context(tc.tile_pool(name="pin", bufs=2))
        ap = ctx.enter_context(tc.tile_pool(name="ap", bufs=2))
        pps = ctx.enter_context(tc.tile_pool(name="pps", bufs=2, space="PSUM"))
        stp = ctx.enter_context(tc.tile_pool(name="stp", bufs=2, space="PSUM"))
        ovp = ctx.enter_context(tc.tile_pool(name="ovp", bufs=2, space="PSUM"))
        yp = ctx.enter_context(tc.tile_pool(name="yp", bufs=2, space="PSUM"))
        dsc = ctx.enter_context(tc.tile_pool(name="dsc", bufs=3, space="DRAM"))

        # persistent tensors
        q_heads = [res.tile([128, npad], BF16, tag=f"qT{h}", name=f"qT{h}")
                   for h in range(HPC)]
        k_heads = [res.tile([128, M], BF16, tag=f"kT{h}", name=f"kT{h}")
                   for h in range(HPC)]
        v_sb = res.tile([128, M_CH, HPC * VAUG], BF16, tag="v_sb", name="v_sb")
        wk_sb = res.tile([128, DK_CH, HPC * CP], BF16, tag="wk_sb", name="wk_sb")
        wv_sb = res.tile([128, DK_CH, HPC * VAUG], BF16, tag="wv_sb", name="wv_sb")
        wq_sb = res.tile([128, DQ_CH, HPC * CP], BF16, tag="wq_sb", name="wq_sb")
        wout_sb = res.tile([80, HPC, QD], BF16, tag="wout_sb", name="wout_sb")
        bias_bc = res.tile([128, QD], F32, tag="bias_bc", name="bias_bc")

        # weight/bias loads — all on the sync queue, in need order, so the
        # DMA engine transfers wk+ksl0 first and the first matmul starts
        # ASAP (a second queue would race its transfers ahead of ksl0).
        nc.sync.dma_start(out=wk_sb[:], in_=wk[:])
        ksls = []
        for mg in range(2):
            ksl = pin.tile([128, DK_CH, 512], BF16, tag="ksl", name="ksl")
            nc.sync.dma_start(out=ksl[:], in_=kt[:, mg])
            ksls.append(ksl)
            if mg == 0:
                nc.sync.dma_start(out=wv_sb[:], in_=wv[:])
                nc.sync.dma_start(out=wq_sb[:], in_=wq[:])
        nc.sync.dma_start(out=wout_sb[:], in_=wout[:])
        nc.sync.dma_start(out=bias_bc[:], in_=_pbcast(brow[0:1, :], 128))

        # ---- k projection: kT[h] = (key @ Wk_h)^T ----
        for mg in range(2):
            ksl = ksls[mg]
            if mg == 1:
                # v slices (two 4-chunk slabs) land behind the k chain
                vsls = []
                for vg in range(2):
                    vsl = pin.tile([128, 4, DK_CH, 128], BF16, tag="vsl",
                                   name="vsl")
                    nc.sync.dma_start(out=vsl[:], in_=vt[:, vg * 4:vg * 4 + 4])
                    vsls.append(vsl)
            for h in range(HPC):
                ps = pps.tile([128, 512], F32, tag="pp", name="pp")
                for dc in range(DK_CH):
                    nc.tensor.matmul(
                        ps[:], wk_sb[:, dc, h * CP:(h + 1) * CP],
                        ksl[:, dc, :],
                        start=(dc == 0), stop=(dc == DK_CH - 1))
                nc.scalar.activation(
                    k_heads[h][:, mg * 512:(mg + 1) * 512], ps[:], COPY)

        # ---- v projection: natural [m, head-aug channels] ----
        for mc in range(M_CH):
            vsl = vsls[mc // 4]
            ps = pps.tile([128, 512], F32, tag="pp", name="pp")
            for dc in range(DK_CH):
                nc.tensor.matmul(
                    ps[:, :HPC * VAUG], vsl[:, mc % 4, dc, :], wv_sb[:, dc, :],
                    start=(dc == 0), stop=(dc == DK_CH - 1))
            nc.scalar.activation(v_sb[:, mc, :], ps[:, :HPC * VAUG], COPY)
        # softmax-denominator ones column per head (after all v copies)
        for h in range(HPC):
            nc.vector.tensor_copy(
                v_sb[:, :, h * VAUG + 96:h * VAUG + 97],
                nc.const_aps.tensor(1.0, (128, M_CH, 1), F32))

        # ---- per query group: q-proj, attention, out-proj ----
        # q-proj runs one group ahead of attention (software pipeline) so
        # its matmuls fill PE gaps while ACT paces the previous group.
        def qproj(gi, g0, gsz):
            xq = pin.tile([128, DQ_CH, 512], BF16, tag="xq", name="xq")
            nc.sync.dma_start(out=xq[:, :, :gsz], in_=xt[:, gi, :, :gsz])
            for h in range(HPC):
                ps = pps.tile([128, 512], F32, tag="pp", name="pp")
                for dc in range(DQ_CH):
                    nc.tensor.matmul(
                        ps[:, :gsz], wq_sb[:, dc, h * CP:(h + 1) * CP],
                        xq[:, dc, :gsz],
                        start=(dc == 0), stop=(dc == DQ_CH - 1))
                nc.vector.tensor_copy(q_heads[h][:, g0:g0 + gsz], ps[:, :gsz])

        qproj(0, *groups[0])
        for gi, (g0, gsz) in enumerate(groups):
            # attention: unnormalized out + denominator per head, then one
            # batched reciprocal round-trip + partition-broadcast per group
            # (per-head chains for the last group, to shorten the exposed
            # tail after the final softmax-exp)
            last = gi == len(groups) - 1
            bcast = ap.tile([80, HPC, 512], BF16, tag="bcast", name="bcast")
            rdr = dsc.tile([HPC, 512], BF16, tag="rdr", name="rdr")
            o_uns = []
            for h in range(HPC):
                expst = ap.tile([128, M_CH, 512], BF16, tag="expst",
                                name="expst")
                for mc in range(M_CH):
                    st = stp.tile([128, 512], F32, tag="st", name="st")
                    nc.tensor.matmul(
                        st[:, :gsz], k_heads[h][:, mc * 128:(mc + 1) * 128],
                        q_heads[h][:, g0:g0 + gsz], start=True, stop=True)
                    nc.scalar.activation(expst[:, mc, :gsz], st[:, :gsz],
                                         EXP, scale=SCALE)
                oaug = ovp.tile([VAUG, 512], F32, tag="oaug", name="oaug")
                for mc in range(M_CH):
                    nc.tensor.matmul(
                        oaug[:, :gsz], v_sb[:, mc, h * VAUG:(h + 1) * VAUG],
                        expst[:, mc, :gsz],
                        start=(mc == 0), stop=(mc == M_CH - 1))
                o_un = ap.tile([80, 512], F32, tag=f"oun{h}", name=f"oun{h}")
                nc.vector.tensor_copy(o_un[:, :gsz], oaug[:80, :gsz])
                recip_h = ap.tile([1, 512], BF16, tag=f"recip{h}",
                                  name=f"recip{h}")
                with nc.allow_low_precision(reason="bf16 softmax recip"):
                    nc.vector.reciprocal(recip_h[:, :gsz],
                                         oaug[96:97, :gsz])
                o_uns.append(o_un)
                nc.sync.dma_start(out=rdr[h:h + 1, :gsz],
                                  in_=recip_h[:, :gsz])
                if last:
                    nc.sync.dma_start(out=bcast[:, h, :gsz],
                                      in_=_pbcast(rdr[h:h + 1, :gsz], 80))
            if not last:
                nc.sync.dma_start(out=bcast[:, :, :gsz],
                                  in_=_pbcast(rdr[:, :gsz], 80))

            if gi + 1 < len(groups):
                qproj(gi + 1, *groups[gi + 1])

            out_hs = []
            for h in range(HPC):
                out_h = ap.tile([80, 512], BF16, tag=f"outH{h}",
                                name=f"outH{h}")
                nc.vector.tensor_tensor(
                    out_h[:, :gsz], o_uns[h][:80, :gsz], bcast[:, h, :gsz],
                    MUL)
                out_hs.append(out_h)

            ntile = gsz // 128
            ysb = ap.tile([128, 4, QD], BF16, tag="ysb", name="ysb")
            for nt0 in range(ntile):
                for di in range(2):
                    yps = yp.tile([128, 512], F32, tag="y", name="y")
                    for h in range(HPC):
                        nc.tensor.matmul(
                            yps[:, :320],
                            out_hs[h][:, nt0 * 128:(nt0 + 1) * 128],
                            wout_sb[:, h, di * 320:(di + 1) * 320],
                            start=(h == 0), stop=(h == HPC - 1))
                    nc.vector.tensor_tensor(
                        ysb[:, nt0, di * 320:(di + 1) * 320], yps[:, :320],
                        bias_bc[:, di * 320:(di + 1) * 320], ADD)
                if last:
                    # per-qtile stores so only the last 1/4 is tail-exposed
                    nc.sync.dma_start(out=y[:, gi, nt0:nt0 + 1, :],
                                      in_=ysb[:, nt0:nt0 + 1, :])
            if not last:
                nc.sync.dma_start(out=y[:, gi, :ntile, :],
                                  in_=ysb[:, :ntile, :])


def _pm(a, nchunk, p=128):
    """[nchunk*p, F...] -> partition-major [p, nchunk, F...]."""
    return np.ascontiguousarray(
        a.reshape(nchunk, p, *a.shape[1:]).transpose(
            1, 0, *range(2, a.ndim + 1)))


def _prep_core_inputs(x, key, value, wq, wk, wv, wout, bout,
                      qmask_idx, npad):
    """Host-side shard prep: returns list of 8 in_maps (bf16, partition-
    major layouts matching the DRAM tensor declarations in build())."""
    ngr = -(-npad // 512)
    in_maps = []
    xt_b, kt_b, vt_b = {}, {}, {}
    for b in range(B):
        idx = qmask_idx[b]
        xs = np.zeros((QD, ngr * 512), dtype=bfloat16)
        xs[:, :len(idx)] = np.ascontiguousarray(x[b][idx].T).astype(bfloat16)
        # [640, ngr*512] -> [5, 128, ngr, 512] -> [128, ngr, 5, 512]
        xt_b[b] = np.ascontiguousarray(
            xs.reshape(DQ_CH, 128, ngr, 512).transpose(1, 2, 0, 3))
        kb = np.ascontiguousarray(key[b].T).astype(bfloat16)
        kt_b[b] = np.ascontiguousarray(
            kb.reshape(DK_CH, 128, 2, 512).transpose(1, 2, 0, 3))
        vb = np.ascontiguousarray(value[b].T).astype(bfloat16)
        vt_b[b] = np.ascontiguousarray(
            vb.reshape(DK_CH, 128, M_CH, 128).transpose(1, 2, 0, 3))

    w_half = {}
    for hh in range(2):
        wq_a = np.zeros((QD, HPC * CP), dtype=bfloat16)
        wk_a = np.zeros((KD, HPC * CP), dtype=bfloat16)
        wv_a = np.zeros((KD, HPC * VAUG), dtype=bfloat16)
        for hp in range(HPC):
            hg = hh * HPC + hp
            wq_a[:, hp * CP:hp * CP + 80] = wq[:, hg * 80:(hg + 1) * 80].astype(bfloat16)
            wk_a[:, hp * CP:hp * CP + 80] = wk[:, hg * 80:(hg + 1) * 80].astype(bfloat16)
            wv_a[:, hp * VAUG:hp * VAUG + 80] = wv[:, hg * 80:(hg + 1) * 80].astype(bfloat16)
        w_half[hh] = (_pm(wq_a, DQ_CH), _pm(wk_a, DK_CH), _pm(wv_a, DK_CH))

    for core in range(8):
        b, hh = core // 2, core % 2
        wq_a, wk_a, wv_a = w_half[hh]
        wo = wout[hh * HPC * 80:(hh + 1) * HPC * 80].astype(bfloat16)
        wout_a = np.ascontiguousarray(
            wo.reshape(HPC, 80, QD).transpose(1, 0, 2))
        brow_a = (bout if hh == 0 else np.zeros_like(bout)).astype(np.float32)
        in_maps.append({
            "xt": xt_b[b], "kt": kt_b[b], "vt": vt_b[b],
            "wq": wq_a, "wk": wk_a, "wv": wv_a, "wout": wout_a,
            "brow": brow_a.reshape(1, QD),
        })
    return in_maps


def kernel(x, key, value, box_mask, Wq, Wk, Wv, Wout, bout, _trace=False):
    x = np.asarray(x, dtype=np.float32)
    key = np.asarray(key, dtype=np.float32)
    value = np.asarray(value, dtype=np.float32)
    box_mask = np.asarray(box_mask, dtype=np.float32)
    Wq, Wk, Wv = (np.asarray(a, dtype=np.float32) for a in (Wq, Wk, Wv))
    Wout = np.asarray(Wout, dtype=np.float32)
    bout = np.asarray(bout, dtype=np.float32)

    qmask = box_mask[:, 0].reshape(B, N) > 0.5
    qmask_idx = [np.nonzero(qmask[b])[0] for b in range(B)]
    cnt_max = max(1, max(len(i) for i in qmask_idx))
    npad = -(-cnt_max // 128) * 128

    nc = build(npad)
    in_maps = _prep_core_inputs(x, key, value, Wq, Wk, Wv, Wout, bout,
                                qmask_idx, npad)
    kr = run_bass_kernel_spmd(nc, in_maps, core_ids=list(range(8)),
                              trace=_trace)
    results = kr.results

    out = np.broadcast_to(bout, (B, N, QD)).copy().astype(np.float32)
    for b in range(B):
        idx = qmask_idx[b]
        # y is [128, ngr, 4, QD]: query g*512 + nt0*128 + p lives at
        # y[p, g, nt0]
        y0 = results[2 * b]["y"].transpose(1, 2, 0, 3).reshape(-1, QD)
        y1 = results[2 * b + 1]["y"].transpose(1, 2, 0, 3).reshape(-1, QD)
        out[b][idx] = (y0[:len(idx)].astype(np.float32)
                       + y1[:len(idx)].astype(np.float32))
    if _trace:
        return out, kr
    return out
